# revision 46
# baseline (speedup 1.0000x reference)
"""BGCF layer forward on 8 Trainium2 NeuronCores (Bass/Tile).

Strategy (v5): each core computes, for its shard, the raw aggregation
sums (all O(N^2) contractions) on device; the host applies the tiny
O(N*D^2) per-row normalizations/projections (degree scaling, 64x64 W
matmuls, tanh, l2norm, index gathers) during unsharding:
  - score pass: s = ue_b @ ie.T in fp8 DoubleRow (iet relaid out to
    128 partitions), masked softmax numerators/denominator via
    exp + 30*adj trick (groups 0..5 mask on DVE, 6..7 on PE by
    accumulating 30*adj into the score psum and biasing the exp).
  - gather pass: si (coef numerator + denominator via ones column),
    au (adjacency row sums), ou (obs rows) accumulate in one psum bank.
  - item pass: 256 sampled columns per core, full 8192-user
    contraction, fp8 hi+residual DoubleRow.
Cost-model-aware scheduling: three DMA queues (SP/Pool/ACT) carry
~13us each; ACT's DMA rides BEFORE its exp chain (the scheduling pass
would freeze late fillers ahead of the exps); adjacency columns load
early/assorted so PE's mid-kernel holes absorb the column matmuls; all
engines are kept busy across producer boundaries to dodge the
idle-wakeup penalty (+1717ns on DMA edges).
Outputs (raw sums): HB [128,3,65] = si|den, au, ou; ITAO [128,4,64].
"""
import math
import numpy as np
import ml_dtypes

import concourse.bacc as bacc
import concourse.tile as tile
import concourse.mybir as mybir
from concourse.bass_utils import run_bass_kernel_spmd

F32 = mybir.dt.float32
BF16 = mybir.dt.bfloat16
FP8 = mybir.dt.float8e4
ACT_F = mybir.ActivationFunctionType
DR = mybir.MatmulPerfMode.DoubleRow

NP_BF16 = ml_dtypes.bfloat16
NP_FP8 = ml_dtypes.float8_e4m3

M = 8            # cores
U = 8192         # users
I = 8192         # items
D = 64
B = 1024         # batch
BSH = B // M     # batch rows per core (128)
SEL = 2048       # padded sampled item columns
SELC = SEL // M  # sampled columns per core (256)
NCH = U // 128   # user-row chunks (64)
NT = 64          # item tiles
EPS = 1e-6
AMP = 30.0       # adjacency amplitude (exact in fp8e4m3)
EXPB = -AMP + math.log(AMP)   # exp bias for PE-masked groups
NG = 8           # score/exp groups
GT = NT // NG    # tiles per group (8)
PE_MASK_G = (6, 7)   # groups masked via PE bias instead of DVE mul

OUT_NAMES = ("HB", "ITAO")

_CACHE = {}


def _build():
    nc = bacc.Bacc("TRN2", target_bir_lowering=False, debug=False, num_devices=M)

    # ---- inputs (per core) ----
    IET3 = nc.dram_tensor("IET3", [64, 2, 4096], FP8, kind="ExternalInput")
    UEGT3 = nc.dram_tensor("UEGT3", [64, 2, 128], FP8, kind="ExternalInput")
    IEA = nc.dram_tensor("IEA", [128, NT, D + 1], BF16, kind="ExternalInput")
    UEAHR = nc.dram_tensor("UEAHR", [128, NCH // 2, 2, 2 * D], FP8, kind="ExternalInput")
    AUTD = nc.dram_tensor("AUTD", [128, NT, BSH], FP8, kind="ExternalInput")   # 30*adj
    OUTD = nc.dram_tensor("OUTD", [128, NT, BSH], FP8, kind="ExternalInput")
    ACTD = nc.dram_tensor("ACTD", [128, NCH // 2, 2, SELC], FP8, kind="ExternalInput")
    OCTD = nc.dram_tensor("OCTD", [128, NCH // 2, 2, SELC], FP8, kind="ExternalInput")
    IDN = nc.dram_tensor("IDN", [128, 128], BF16, kind="ExternalInput")

    # ---- outputs (raw sums; host normalizes/projects) ----
    HB = nc.dram_tensor("HB", [BSH, 3, D + 1], F32, kind="ExternalOutput")
    ITAO = nc.dram_tensor("ITAO", [128, 4, D], F32, kind="ExternalOutput")

    with tile.TileContext(nc) as tc:
        with tc.tile_pool(name="pscore", bufs=3, space="PSUM") as pscore, \
             tc.tile_pool(name="psacc", bufs=1, space="PSUM") as psacc, \
             tc.tile_pool(name="pscol", bufs=1, space="PSUM") as pscol, \
             tc.tile_pool(name="per", bufs=1) as per, \
             tc.tile_pool(name="stp", bufs=4) as stp:

            # ---- persistent SBUF tiles ----
            iet_sb = per.tile([64, 2, 4096], FP8, tag="iet")
            uegt_sb = per.tile([64, 2, 128], FP8, tag="uegt")
            iea_sb = per.tile([128, NT, D + 1], BF16, tag="iea")
            ueahr_sb = per.tile([128, NCH // 2, 2, 2 * D], FP8, tag="ueahr")
            aut_sb = per.tile([128, NT, BSH], FP8, tag="aut")
            out_sb = per.tile([128, NT, BSH], FP8, tag="outd")
            act_sb = per.tile([128, NCH // 2, 2, SELC], FP8, tag="actd")
            oct_sb = per.tile([128, NCH // 2, 2, SELC], FP8, tag="octd")
            idn_sb = per.tile([128, 128], BF16, tag="idn")
            hb_sb = per.tile([128, 3, D + 1], F32, tag="hb")
            itao_sb = per.tile([128, 4, D], F32, tag="itao")
            warm_sb = per.tile([128, 128], BF16, tag="warm")
            expb_sb = per.tile([128, 1], F32, tag="expb")

            # ---- PSUM accumulators: one pool (bank) per open group ----
            ps_sia = psacc.tile([128, 3, D + 1], F32, tag="sia")   # si|au|ou
            ps_ou = ps_sia[:, 2, 0:D]
            ps_c4 = pscol.tile([128, 4, D], F32, tag="c4")  # item A|O cols
            ps_ca = ps_c4[:, 0:2, :]
            ps_co = ps_c4[:, 2:4, :]

            # ---- SP queue ----
            nc.sync.dma_start(iet_sb[:, :, 0:2048], IET3[:, :, 0:2048])
            nc.sync.dma_start(iet_sb[:, :, 2048:4096], IET3[:, :, 2048:4096])
            nc.sync.dma_start(act_sb[:, 0:8, :, :], ACTD[:, 0:8, :, :])
            nc.sync.dma_start(iea_sb[:, 0:32, :], IEA[:, 0:32, :])
            nc.sync.dma_start(act_sb[:, 8:16, :, :], ACTD[:, 8:16, :, :])
            nc.sync.dma_start(iea_sb[:, 32:64, :], IEA[:, 32:64, :])
            nc.sync.dma_start(act_sb[:, 16:32, :, :], ACTD[:, 16:32, :, :])
            nc.sync.dma_start(oct_sb[:, 20:26, :, :], OCTD[:, 20:26, :, :])

            # ---- Pool queue ----
            nc.gpsimd.dma_start(uegt_sb[:], UEGT3[:, :, :])
            nc.gpsimd.dma_start(idn_sb[:], IDN[:, :])
            nc.gpsimd.dma_start(aut_sb[:, 0:16, :], AUTD[:, 0:16, :])
            nc.gpsimd.dma_start(ueahr_sb[:, 0:16, :, :], UEAHR[:, 0:16, :, :])
            nc.gpsimd.dma_start(oct_sb[:, 0:8, :, :], OCTD[:, 0:8, :, :])
            nc.gpsimd.dma_start(aut_sb[:, 16:32, :], AUTD[:, 16:32, :])
            nc.gpsimd.dma_start(oct_sb[:, 8:16, :, :], OCTD[:, 8:16, :, :])
            nc.gpsimd.dma_start(aut_sb[:, 32:64, :], AUTD[:, 32:64, :])
            nc.gpsimd.dma_start(out_sb[:, 0:32, :], OUTD[:, 0:32, :])
            nc.gpsimd.dma_start(ueahr_sb[:, 16:32, :, :], UEAHR[:, 16:32, :, :])
            nc.gpsimd.dma_start(oct_sb[:, 16:20, :, :], OCTD[:, 16:20, :, :])

            # ---- ACT queue: early loads before the exp chain ----
            nc.scalar.dma_start(out_sb[:, 32:64, :], OUTD[:, 32:64, :])
            nc.scalar.dma_start(oct_sb[:, 26:32, :, :], OCTD[:, 26:32, :, :])

            # ---- engine bridges + bank opening ----
            nc.vector.memset(warm_sb[:], 0.0)
            nc.vector.memset(expb_sb[:], EXPB)
            dvebridge_sb = per.tile([128, 3700], BF16, tag="dveb")
            nc.vector.memset(dvebridge_sb[:], 0.0)
            ps_warm = ps_co[0:64, 0, :]
            for _ in range(13):
                nc.tensor.matmul(ps_warm, warm_sb[:, 0:64], warm_sb[:, 0:64],
                                 start=True, stop=True, skip_group_check=True)
            # open accumulator banks: zero-touch every region once (first
            # mm of each bank start=True); later accumulations start=False
            for k in range(3):
                nc.tensor.matmul(ps_sia[:, k, :], warm_sb[:],
                                 warm_sb[:, 0:D + 1], start=(k == 0),
                                 stop=False, skip_group_check=True)
            for k in range(4):
                nc.tensor.matmul(ps_c4[:, k, :], warm_sb[:],
                                 warm_sb[:, 0:D], start=(k == 0), stop=False,
                                 skip_group_check=True)

            # ---- score pass ----
            score_tiles = {}

            def score_mms(g):
                pe_mask = g in PE_MASK_G
                ps_s = pscore.tile([128, GT * BSH], F32, tag="s")
                for j in range(GT):
                    t = g * GT + j
                    b, sl = (0, t) if t < 32 else (1, t - 32)
                    nc.tensor.matmul(ps_s[:, j * BSH:(j + 1) * BSH],
                                     iet_sb[32 * b:32 * b + 32, :,
                                            sl * 128:(sl + 1) * 128],
                                     uegt_sb[32 * b:32 * b + 32, :, :],
                                     start=True, stop=not pe_mask,
                                     perf_mode=DR, skip_group_check=True)
                    if pe_mask:
                        nc.tensor.matmul(ps_s[:, j * BSH:(j + 1) * BSH],
                                         idn_sb[:], aut_sb[:, t, :],
                                         start=False, stop=True,
                                         skip_group_check=True)
                score_tiles[g] = ps_s

            def exp_g(g):
                """For PE-masked groups the exp output IS stm (30*exp*adj)."""
                ps_s = score_tiles.pop(g)
                st = stp.tile([128, GT, BSH], BF16, tag="st")
                if g in PE_MASK_G:
                    nc.scalar.activation(st[:].rearrange("p a b -> p (a b)"),
                                         ps_s[:], ACT_F.Exp, bias=expb_sb[:])
                else:
                    nc.scalar.activation(st[:].rearrange("p a b -> p (a b)"),
                                         ps_s[:], ACT_F.Exp)
                return st

            def mul_g(g, st):
                stm = stp.tile([128, GT, BSH], BF16, tag="stm")
                nc.vector.tensor_mul(stm[:], st[:],
                                     aut_sb[:, g * GT:(g + 1) * GT, :])
                return stm

            def gather_mms(g, stm):
                for j in range(GT):
                    t = g * GT + j
                    nc.tensor.matmul(ps_sia[:, 0, :], stm[:, j, :],
                                     iea_sb[:, t, :],
                                     start=False, stop=(t == NT - 1),
                                     skip_group_check=True)
                    nc.tensor.matmul(ps_sia[:, 1, 0:D], aut_sb[:, t, :],
                                     iea_sb[:, t, 0:D],
                                     start=False, stop=(t == NT - 1),
                                     skip_group_check=True)

            def ou_mms(t0, t1):
                for t in range(t0, t1):
                    nc.tensor.matmul(ps_ou, out_sb[:, t, :],
                                     iea_sb[:, t, 0:D],
                                     start=False, stop=(t == NT - 1),
                                     skip_group_check=True)

            def col_pass(mat_sb, ps_c, u0, u1):
                for tt in range(SELC // 128):
                    for u in range(u0, u1):
                        last = (u == NCH // 2 - 1)
                        nc.tensor.matmul(ps_c[:, tt, :],
                                         mat_sb[:, u, :,
                                                tt * 128:(tt + 1) * 128],
                                         ueahr_sb[:, u, :, 0:D],
                                         start=False, stop=False,
                                         perf_mode=DR, skip_group_check=True)
                        nc.tensor.matmul(ps_c[:, tt, :],
                                         mat_sb[:, u, :,
                                                tt * 128:(tt + 1) * 128],
                                         ueahr_sb[:, u, :, D:2 * D],
                                         start=False, stop=last,
                                         perf_mode=DR, skip_group_check=True)

            # ================= emission schedule =================
            sts = {}
            stms = {}

            def stage(g):
                sts[g] = exp_g(g)
                if g in PE_MASK_G:
                    stms[g] = sts[g]
                else:
                    stms[g] = mul_g(g, sts[g])

            score_mms(0)
            score_mms(1)
            stage(0)
            score_mms(2)
            stage(1)
            score_mms(3)
            gather_mms(0, stms[0])
            stage(2)
            score_mms(4)
            gather_mms(1, stms[1])
            col_pass(oct_sb, ps_co, 0, 4)
            stage(3)
            score_mms(5)
            gather_mms(2, stms[2])
            col_pass(act_sb, ps_ca, 0, 8)
            stage(4)
            score_mms(6)
            gather_mms(3, stms[3])
            col_pass(oct_sb, ps_co, 4, 12)
            stage(5)
            score_mms(7)
            gather_mms(4, stms[4])
            col_pass(act_sb, ps_ca, 8, 16)
            ou_mms(0, 32)
            stage(6)
            gather_mms(5, stms[5])
            col_pass(oct_sb, ps_co, 12, 20)
            ou_mms(32, NT)
            stage(7)
            gather_mms(6, stms[6])
            col_pass(act_sb, ps_ca, 16, 24)
            gather_mms(7, stms[7])
            col_pass(oct_sb, ps_co, 20, 26)
            col_pass(act_sb, ps_ca, 24, 32)

            # user/obs raw sums out (split: ou region can complete early)
            nc.scalar.activation(hb_sb[:, 2, :], ps_sia[:, 2, :], ACT_F.Copy)
            nc.scalar.activation(hb_sb[:, 0:2, :], ps_sia[:, 0:2, :], ACT_F.Copy)
            nc.sync.dma_start(HB[:, :, :], hb_sb[:])

            # item-A raw sums out
            nc.scalar.activation(itao_sb[:, 0:2, :], ps_ca[:], ACT_F.Copy)

            col_pass(oct_sb, ps_co, 26, 32)
            nc.scalar.activation(itao_sb[:, 2:4, :], ps_co[:], ACT_F.Copy)
            nc.scalar.dma_start(ITAO[:, :, :], itao_sb[:])

    nc.compile()
    return nc


def _get_nc():
    if "nc" not in _CACHE:
        _CACHE["nc"] = _build()
    return _CACHE["nc"]


def _pmaj(x, inner):
    """[8192, inner] row-major -> [128, 64, inner] partition-major."""
    return np.ascontiguousarray(
        x.reshape(NCH, 128, inner).transpose(1, 0, 2))


def _pairmaj(x, inner):
    """[8192, inner] -> [128, 32, 2, inner], pairs along rows."""
    return np.ascontiguousarray(
        x.reshape(NCH // 2, 128, 2, inner).transpose(1, 0, 2, 3))


def _prep_in_maps(users, pos_items, neg_items, adj_matrix, obs_users,
                  obs_pos_items, obs_neg_items, obs_adj_matrix, user_emb,
                  item_emb, W_1, W_2, W_obs):
    adj = np.asarray(adj_matrix, dtype=np.float32)
    oadj = np.asarray(obs_adj_matrix, dtype=np.float32)
    ue = np.asarray(user_emb, dtype=np.float32)
    ie = np.asarray(item_emb, dtype=np.float32)
    users = np.asarray(users).astype(np.int64)
    obs_users = np.asarray(obs_users).astype(np.int64)

    # sampled item columns (padded to SEL)
    pn = np.concatenate([np.asarray(pos_items), np.asarray(neg_items)])
    sel_a, inv_a = np.unique(pn.astype(np.int64), return_inverse=True)
    selp_a = np.zeros(SEL, np.int64)
    selp_a[:len(sel_a)] = sel_a
    on = np.concatenate([np.asarray(obs_pos_items), np.asarray(obs_neg_items)])
    sel_o, inv_o = np.unique(on.astype(np.int64), return_inverse=True)
    selp_o = np.zeros(SEL, np.int64)
    selp_o[:len(sel_o)] = sel_o

    adj_cols = adj[:, selp_a]          # [8192, 2048]
    oadj_cols = oadj[:, selp_o]

    ones_u = np.ones((U, 1), np.float32)
    iea = _pmaj(np.concatenate([ie, ones_u], axis=1), D + 1).astype(NP_BF16)

    # x64 scaling keeps the fp8 residual out of the subnormal range
    ue_s = ue * 64.0
    uea_hi = ue_s.astype(NP_FP8)
    uea_res = (ue_s - uea_hi.astype(np.float32)).astype(NP_FP8)
    ueahr = np.concatenate(
        [_pairmaj(uea_hi.astype(np.float32), D),
         _pairmaj(uea_res.astype(np.float32), D)], axis=3).astype(NP_FP8)

    # IET3: ie.T fp8, DR pairs (d = 2k+i); 3 partition blocks at legal
    # bases {0,32,64} holding tiles [0:22), [22:43), [43:64)
    iet_t = np.ascontiguousarray(ie.T).astype(NP_FP8)        # [64, 8192]
    ietr = iet_t.reshape(32, 2, 64, 128)                     # [k, i, t, m]
    iet3 = np.ascontiguousarray(
        ietr.reshape(32, 2, 2, 32, 128).transpose(2, 0, 1, 3, 4)
    ).reshape(64, 2, 4096)

    idn = np.eye(128, dtype=np.float32).astype(NP_BF16)

    in_maps = []
    meta = []
    for c in range(M):
        bs = slice(c * BSH, (c + 1) * BSH)
        ub = users[bs]
        ob = obs_users[bs]
        cs = slice(c * SELC, (c + 1) * SELC)
        uegt = np.ascontiguousarray(ue[ub].T).astype(NP_FP8).reshape(32, 2, BSH)
        in_maps.append({
            "IET3": iet3,
            "UEGT3": np.ascontiguousarray(
                np.broadcast_to(uegt[None], (2, 32, 2, BSH))).reshape(64, 2, BSH),
            "IEA": iea,
            "UEAHR": ueahr,
            "AUTD": _pmaj(np.ascontiguousarray(adj[ub].T) * AMP, BSH).astype(NP_FP8),
            "OUTD": _pmaj(np.ascontiguousarray(oadj[ob].T), BSH).astype(NP_FP8),
            "ACTD": _pairmaj(np.ascontiguousarray(adj_cols[:, cs]), SELC).astype(NP_FP8),
            "OCTD": _pairmaj(np.ascontiguousarray(oadj_cols[:, cs]), SELC).astype(NP_FP8),
            "IDN": idn,
        })
        meta.append({
            "deg_u": adj[ub].sum(axis=1),
            "odeg_u": oadj[ob].sum(axis=1),
        })
    dega = 1.0 / (64.0 * (adj_cols.sum(axis=0) + EPS))        # [2048]
    dego = 1.0 / (64.0 * (oadj_cols.sum(axis=0) + EPS))
    return in_maps, (inv_a, inv_o, meta, dega, dego)


def _assemble(results, aux, W_1, W_2, W_obs):
    inv_a, inv_o, meta, dega, dego = aux
    W_1 = np.asarray(W_1, np.float32)
    W_2 = np.asarray(W_2, np.float32)
    W_obs = np.asarray(W_obs, np.float32)

    h1_l, h2u_l, obsu_l = [], [], []
    for c, r in enumerate(results):
        hb = np.asarray(r["HB"]).reshape(BSH, 3, D + 1)
        num, den = hb[:, 0, 0:D], hb[:, 0, D]
        au, ou = hb[:, 1, 0:D], hb[:, 2, 0:D]
        h1_l.append((num / (den + AMP * EPS)[:, None]) @ W_1)
        h2u_l.append((au / (AMP * (meta[c]["deg_u"] + EPS))[:, None]) @ W_2)
        obsu_l.append((ou / (meta[c]["odeg_u"] + EPS)[:, None]) @ W_obs)
    h1 = np.concatenate(h1_l, axis=0)
    h2u = np.concatenate(h2u_l, axis=0)
    obsu = np.tanh(np.concatenate(obsu_l, axis=0))

    def unpack(x, k0):
        x = np.asarray(x).reshape(128, 4, D)
        return x[:, k0:k0 + 2, :].transpose(1, 0, 2).reshape(SELC, D)

    ita_raw = np.concatenate([unpack(r["ITAO"], 0) for r in results], axis=0)
    ito_raw = np.concatenate([unpack(r["ITAO"], 2) for r in results], axis=0)
    ita = (ita_raw * dega[:, None]) @ W_2
    ito = np.tanh((ito_raw * dego[:, None]) @ W_obs)

    h2_pos = ita[inv_a[:B]]
    h2_neg = ita[inv_a[B:]]
    obs_pos = ito[inv_o[:B]]
    obs_neg = ito[inv_o[B:]]

    def l2n(x):
        n = np.sqrt((x * x).sum(axis=1, keepdims=True))
        return x / np.maximum(n, 1e-12)

    h_user = np.tanh(np.concatenate([h1, h2u, obsu], axis=1))
    h_pos = np.tanh(np.concatenate([h2_pos, h2_pos, obs_pos], axis=1))
    h_neg = np.tanh(np.concatenate([h2_neg, h2_neg, obs_neg], axis=1))
    return l2n(h_user), l2n(h_pos), l2n(h_neg)


def kernel(users, pos_items, neg_items, adj_matrix, obs_users, obs_pos_items,
           obs_neg_items, obs_adj_matrix, iteration, user_emb, item_emb,
           W_1, W_2, W_obs):
    nc = _get_nc()
    in_maps, aux = _prep_in_maps(
        users, pos_items, neg_items, adj_matrix, obs_users, obs_pos_items,
        obs_neg_items, obs_adj_matrix, user_emb, item_emb, W_1, W_2, W_obs)
    res = run_bass_kernel_spmd(nc, in_maps, core_ids=list(range(M)))
    return _assemble(res.results, aux, W_1, W_2, W_obs)
